# revision 29
# baseline (speedup 1.0000x reference)
"""Groupwise projection kernel for Trainium2 (8 NeuronCores).

Problem: x [16, 4096, 512] fp32; 8 contiguous token segments per 4096-token
row, each with its own Linear (W [8, 512, 512], b [8, 512]);
out[b, t, :] = x[b, t, :] @ W[g(t)].T + b[g(t)].

Strategy (v13, ~72-74us vs 115-120us fp32r baseline):
  - 16-bit I/O: x and W cast to fp16 on the host, matmul accumulates in
    fp32 PSUM, bias added in fp32, output stored fp16 and upcast on the
    host (rel_err ~5e-4, gate 2e-2). fp16 matmul runs the full-rate
    TensorE path: 8192 tok * 16 blocks * 512 cyc / 2.4 GHz = 54.6us/core;
    HBM traffic ~18.4MB -> ~51us. The kernel sits at the ridge; the
    matmul stream is the critical path, everything else hides under it.
  - Startup: the framework preamble ends ~7us and the first DMA byte
    moves ~8.3us. ALL loads ride the single sync HWDGE ring in exact
    consumption order (w0, c0, bias, c1, ..., w1 after c6, w2 after c8):
    a second active ring makes the SDMA engines round-robin per packet
    and the startup stream crawls, and the scheduler hoists any dep-free
    DMA to its engine's stream head, so ring order is the only reliable
    sequencing. Every x tile is exactly chunk-sized, keeping the SBUF
    destination contiguous (4-8KB descriptors; a strided destination
    shatters into 1KB packets at 1/4 rate). The first two tiles are 256
    tokens so only 768KB gates the first matmul group.
  - HAM warm-up: the PE clock gate sits at 1.2GHz until ~3.4us of
    sustained activity; 22 dummy matmuls on a zeroed scratch tile (N=256,
    tapering to N=128 for a finer drain) bridge PE-busy from the preamble
    end to first-chunk-ready with no idle gap, so the real matmul stream
    runs at 2.4GHz nearly from its first instruction (an idle gap delays
    the un-throttle and the first real tiles then run at 1.2GHz).
  - PSUM->SBUF bias-add alternates DVE / ACT so copies never pace the
    matmuls. Mid-run stores ride the gpsimd SWDGE ring. The last tile
    stores per output block across the rings, and its final block's
    compute+copy is split into two 256-token halves on separate PSUM
    banks (the store stays one 128KB DMA) so only a 256-token copy
    trails the very last matmul.
"""

import sys

sys.path.insert(0, "/opt/trn_rl_repo")

import numpy as np
import concourse.bacc as bacc
import concourse.bass as bass
import concourse.mybir as mybir
import concourse.tile as tile
from concourse.bass_utils import run_bass_kernel_spmd

# run_bass_kernel_spmd imports antenv.axon_hooks when BASS_TRACE is set; some
# images lack that module. Register a no-op fallback so a stray BASS_TRACE
# can only skip profiling, never crash the run.
try:
    import antenv.axon_hooks  # noqa: F401
except ImportError:
    import types

    _hooks = types.ModuleType("antenv.axon_hooks")
    _hooks._hook = None
    _hooks.set_axon_ntff_profile_hook = lambda h: setattr(_hooks, "_hook", h)
    _hooks.get_axon_ntff_profile_hook = lambda: _hooks._hook
    try:
        import antenv

        antenv.axon_hooks = _hooks
        sys.modules["antenv.axon_hooks"] = _hooks
    except ImportError:
        pass

F32 = mybir.dt.float32
F16 = mybir.dt.float16
IDENT = mybir.ActivationFunctionType.Identity

LEN_GROUPS = (256, 512, 768, 384, 640, 512, 576, 448)
NUM_GROUPS, D_IN, D_OUT = 8, 512, 512
BATCH, T = 16, 4096
N_CORES = 8
T_CORE = 8192  # tokens per core (16*4096/8)
KB = D_IN // 128   # 4 contraction blocks
OB = D_OUT // 128  # 4 output blocks
NT = 512           # max moving-dim tile (tokens per matmul)

# Weight slots per core: slot s covers SLOT_SIZES[s] tokens, all of one group.
SLOT_SIZES = (4096, 2560, 1536)
N_SLOTS = 3
# (slot, core) -> group. Tiles the 16*L_g tokens of every group exactly.
SLOT_GROUPS = (
    (0, 1, 1, 2, 2, 2, 6, 7),  # 4096-token slots
    (4, 4, 4, 4, 5, 5, 6, 6),  # 2560-token slots
    (3, 3, 3, 3, 5, 5, 7, 7),  # 1536-token slots
)

# Tiles: small first tiles start compute as early as possible.
TILE_SIZES = [256, 256] + [512] * 15  # 17 tiles, sums to 8192
TILE_STARTS = np.concatenate([[0], np.cumsum(TILE_SIZES)]).tolist()
N_TILES = len(TILE_SIZES)
SLOT_STARTS = np.concatenate([[0], np.cumsum(SLOT_SIZES)]).tolist()

# x staged in chunks; chunk boundaries align with tile boundaries.
CHUNK_SIZES = [256, 256, 512] + [1024] * 6 + [512, 512]
CHUNK_STARTS = np.concatenate([[0], np.cumsum(CHUNK_SIZES)]).tolist()

N_WARM = 22  # dummy warm-up matmuls (N=256 each): bridge PE-busy from the
             # preamble end (~7.6us) to first-chunk-ready (~12.2us) with no
             # idle gap — an idle gap resets the HAM activity window and the
             # first real tiles then run at 1.2GHz

_NC_CACHE = None
_LAST_RESULTS = None  # test harness introspection (exec_time_ns etc.)


def _token_assignment():
    """Per-core global token indices (into x.reshape(-1, 512)), slot-major."""
    starts = np.cumsum((0,) + LEN_GROUPS[:-1])
    pools = []
    for g in range(NUM_GROUPS):
        seg = np.arange(starts[g], starts[g] + LEN_GROUPS[g])
        pools.append(
            (np.arange(BATCH)[:, None] * T + seg[None, :]).reshape(-1)
        )
    used = [0] * NUM_GROUPS
    core_tok = [[] for _ in range(N_CORES)]
    for s in range(N_SLOTS):
        size = SLOT_SIZES[s]
        for c in range(N_CORES):
            g = SLOT_GROUPS[s][c]
            core_tok[c].append(pools[g][used[g]:used[g] + size])
            used[g] += size
    assert all(used[g] == BATCH * LEN_GROUPS[g] for g in range(NUM_GROUPS))
    return [np.concatenate(t) for t in core_tok]


TOKEN_INDEX = _token_assignment()


def _build_nc():
    nc = bacc.Bacc("TRN2", target_bir_lowering=False, debug=False,
                   num_devices=N_CORES)

    # All buffers packed in exact DMA consumption order (sequential HBM).
    xP = nc.dram_tensor("xP", [D_IN * T_CORE], F16, kind="ExternalInput").ap()
    wP = nc.dram_tensor("wP", [N_SLOTS * D_IN * D_OUT], F16,
                        kind="ExternalInput").ap()
    bS = nc.dram_tensor("bS", [128, N_SLOTS * OB], F32,
                        kind="ExternalInput").ap()
    oP = nc.dram_tensor("oP", [D_OUT * T_CORE], F16, kind="ExternalOutput").ap()

    w_len = D_IN * D_OUT

    with tile.TileContext(nc) as tc:
        with (
            tc.tile_pool(name="wpool", bufs=1) as wpool,
            tc.tile_pool(name="bpool", bufs=1) as bpool,
            tc.tile_pool(name="warmp", bufs=1) as warmp,
            tc.tile_pool(name="xpool", bufs=4) as xpool,
            tc.tile_pool(name="opool", bufs=4) as opool,
            tc.tile_pool(name="psum", bufs=8, space=bass.MemorySpace.PSUM) as psum,
        ):
            # Weights resident in SBUF: [p, s, k, o] = W^T[g_s][k*128+p, o]
            w_sb = wpool.tile([128, N_SLOTS, KB, D_OUT], F16)
            b_sb = bpool.tile([128, N_SLOTS * OB], F32)

            # sync ring: w0 then every x chunk, in consumption order. Keep
            # the scalar ring SILENT at startup: a second active ring makes
            # the SDMA engines round-robin per packet and the startup
            # stream crawls (measured 2-5x slowdown on the critical loads).
            nc.sync.dma_start(
                w_sb[:, 0, :, :],
                wP[0:w_len].rearrange("(p k o) -> p k o", p=128, k=KB),
            )

            # HAM warm-up: keep the PE busy while the first loads stream
            # in so the clock gate lifts to 2.4GHz as early as possible.
            # The memset rides gpsimd (the earliest-ready engine) so the
            # dummies start the moment the Tensor engine comes up; the
            # last few dummies are N=128 so the queue drains at a finer
            # granularity once the real data lands.
            warm_sb = warmp.tile([128, 256], F16)
            nc.gpsimd.memset(warm_sb[:], 0.0)
            warm_acc = psum.tile([128, NT], F32, tag="acc")
            for j in range(N_WARM):
                wn = 256 if j < N_WARM - 8 else 128
                nc.tensor.matmul(
                    warm_acc[:, 0:wn], warm_sb[:, 0:128], warm_sb[:, 0:wn],
                    start=True, stop=True,
                )

            x_chunks = [None] * len(CHUNK_SIZES)
            o_off = 0
            for ti in range(N_TILES):
                t0, ntok = TILE_STARTS[ti], TILE_SIZES[ti]
                s = next(
                    j for j in range(N_SLOTS)
                    if SLOT_STARTS[j] <= t0 < SLOT_STARTS[j + 1]
                )
                ci = next(
                    j for j in range(len(CHUNK_SIZES))
                    if CHUNK_STARTS[j] <= t0 < CHUNK_STARTS[j + 1]
                )
                co = t0 - CHUNK_STARTS[ci]  # offset within chunk
                if x_chunks[ci] is None:
                    csz = CHUNK_SIZES[ci]
                    # exact-size tile: contiguous SBUF dest -> 4-8KB DMA
                    # descriptors (a strided dest shatters into 1KB packets)
                    x_sb = xpool.tile(
                        [128, KB, csz], F16,
                        tag=f"x{csz}",
                        bufs=2 if csz < 1024 else 4,
                    )
                    nc.sync.dma_start(
                        x_sb[:],
                        xP[CHUNK_STARTS[ci] * D_IN:CHUNK_STARTS[ci + 1] * D_IN]
                        .rearrange("(p k t) -> p k t", p=128, k=KB),
                    )
                    x_chunks[ci] = x_sb
                    if ci == 0:
                        # bias (128 tiny descriptors) queues behind c0 on
                        # the same ring; it is only needed by the first copy
                        nc.sync.dma_start(b_sb[:], bS)
                    elif ci == 6:
                        # w1 queues once the x stream has a multi-chunk
                        # lead; first needed at tile 9 (~15us later)
                        nc.sync.dma_start(
                            w_sb[:, 1, :, :],
                            wP[w_len:2 * w_len]
                            .rearrange("(p k o) -> p k o", p=128, k=KB),
                        )
                    elif ci == 8:
                        nc.sync.dma_start(
                            w_sb[:, 2, :, :],
                            wP[2 * w_len:3 * w_len]
                            .rearrange("(p k o) -> p k o", p=128, k=KB),
                        )
                x_sb = x_chunks[ci]

                last = ti == N_TILES - 1
                o_sb = opool.tile(
                    [128, OB, ntok], F16, tag=f"o{ntok}",
                    bufs=2 if ntok == 256 else 4,
                )
                o_len = 128 * OB * ntok
                o_dram = oP[o_off:o_off + o_len].rearrange(
                    "(p ob t) -> p ob t", p=128, ob=OB
                )
                o_off += o_len

                # copy engines alternate DVE ("v") / ACT ("s"); on the last
                # tile the final block rides DVE and stores fan per-block
                copy_eng = ("v", "s", "s", "v") if last else ("v", "s", "v", "s")
                store_eng = (nc.gpsimd, nc.gpsimd, nc.sync, nc.scalar)

                for ob in range(OB):
                    bias_ap = b_sb[:, s * OB + ob:s * OB + ob + 1]
                    w_ap = w_sb[:, s, :, ob * 128:(ob + 1) * 128]
                    if last and ob == OB - 1:
                        # split the final block's compute+copy into two
                        # 256-token halves on separate PSUM banks (the first
                        # half's copy hides under the second half's matmuls,
                        # so only a 256-token copy trails the last matmul),
                        # but store the block as one 128KB DMA
                        for h in range(2):
                            ho = h * 256
                            acc = psum.tile([128, NT], F32, tag="acc")
                            for k in range(KB):
                                nc.tensor.matmul(
                                    acc[:, 0:256],
                                    w_ap[:, k, :],
                                    x_sb[:, k, co + ho:co + ho + 256],
                                    start=(k == 0),
                                    stop=(k == KB - 1),
                                )
                            nc.vector.tensor_scalar_add(
                                o_sb[:, ob, ho:ho + 256], acc[:, 0:256],
                                bias_ap,
                            )
                        nc.scalar.dma_start(
                            o_dram[:, ob:ob + 1, :], o_sb[:, ob:ob + 1, :]
                        )
                        continue
                    acc = psum.tile([128, NT], F32, tag="acc")
                    for k in range(KB):
                        nc.tensor.matmul(
                            acc[:, 0:ntok],
                            w_ap[:, k, :],
                            x_sb[:, k, co:co + ntok],
                            start=(k == 0),
                            stop=(k == KB - 1),
                        )
                    if copy_eng[ob] == "v":
                        nc.vector.tensor_scalar_add(
                            o_sb[:, ob, :], acc[:, 0:ntok], bias_ap
                        )
                    else:
                        nc.scalar.activation(
                            o_sb[:, ob, :], acc[:, 0:ntok], IDENT, bias=bias_ap
                        )
                    if last:
                        store_eng[ob].dma_start(
                            o_dram[:, ob:ob + 1, :], o_sb[:, ob:ob + 1, :]
                        )

                if ti == N_TILES - 2:
                    # second-to-last tile: split across gpsimd + sync (the
                    # sync ring is done loading by now; scalar stays free
                    # for the final tile's ACT copies)
                    nc.gpsimd.dma_start(o_dram[:, 0:2, :], o_sb[:, 0:2, :])
                    nc.sync.dma_start(o_dram[:, 2:4, :], o_sb[:, 2:4, :])
                elif not last:
                    # mid-run stores ride the gpsimd SWDGE ring, keeping
                    # compute-dependent instructions off the x ring
                    nc.gpsimd.dma_start(o_dram, o_sb[:])

    nc.compile()
    return nc


def kernel(x: np.ndarray, W: np.ndarray, b: np.ndarray) -> np.ndarray:
    global _NC_CACHE, _LAST_RESULTS
    x = np.asarray(x, dtype=np.float32)
    W = np.asarray(W, dtype=np.float32)
    b = np.asarray(b, dtype=np.float32)

    if _NC_CACHE is None:
        _NC_CACHE = _build_nc()
    nc = _NC_CACHE

    wT = np.ascontiguousarray(W.transpose(0, 2, 1)).astype(np.float16)  # [g,d,o]
    x_flat = x.reshape(BATCH * T, D_IN)

    in_maps = []
    for c in range(N_CORES):
        groups = [SLOT_GROUPS[s][c] for s in range(N_SLOTS)]
        # wP packed [s][p][k][o] = wT[g_s][k*128+p, o]
        wsel = wT[groups]  # [3, 512, 512] = [s, (k p), o]
        wP = np.ascontiguousarray(
            wsel.reshape(N_SLOTS, KB, 128, D_OUT).transpose(0, 2, 1, 3)
        ).reshape(-1)
        # bias laid out [p, s*4 + ob] = b[g_s, ob*128 + p]
        bS = np.ascontiguousarray(
            b[groups].reshape(N_SLOTS, OB, 128).transpose(2, 0, 1)
            .reshape(128, N_SLOTS * OB)
        )
        # xP packed per chunk as [p][k][t]: (p,k,t) = x^T[k*128+p, chunk+t]
        xc = x_flat[TOKEN_INDEX[c]].astype(np.float16)  # [8192, 512]
        parts = []
        for j, csz in enumerate(CHUNK_SIZES):
            t0, t1 = CHUNK_STARTS[j], CHUNK_STARTS[j + 1]
            blk = xc[t0:t1].T  # [512 d, csz]
            parts.append(
                np.ascontiguousarray(
                    blk.reshape(KB, 128, csz).transpose(1, 0, 2)
                ).reshape(-1)
            )
        xP = np.concatenate(parts)
        in_maps.append({"xP": xP, "wP": wP, "bS": bS})

    res = run_bass_kernel_spmd(nc, in_maps, list(range(N_CORES)))
    _LAST_RESULTS = res

    out = np.empty((BATCH * T, D_OUT), dtype=np.float32)
    for c in range(N_CORES):
        oc_flat = res.results[c]["oP"]
        rows = []
        off = 0
        for ntok in TILE_SIZES:
            seg = oc_flat[off:off + 128 * OB * ntok].reshape(128, OB, ntok)
            # [p, ob, t] -> [t, (ob p) = o]
            rows.append(seg.transpose(2, 1, 0).reshape(ntok, D_OUT))
            off += 128 * OB * ntok
        oc = np.concatenate(rows).astype(np.float32)  # [8192, 512]
        out[TOKEN_INDEX[c]] = oc
    return out.reshape(BATCH, T, D_OUT)


# revision 36
# speedup vs baseline: 1.0229x; 1.0229x over previous
"""Groupwise projection kernel for Trainium2 (8 NeuronCores).

Problem: x [16, 4096, 512] fp32; 8 contiguous token segments per 4096-token
row, each with its own Linear (W [8, 512, 512], b [8, 512]);
out[b, t, :] = x[b, t, :] @ W[g(t)].T + b[g(t)].

Strategy (v13, ~72-74us vs 115-120us fp32r baseline):
  - 16-bit I/O: x and W cast to fp16 on the host, matmul accumulates in
    fp32 PSUM, bias added in fp32, output stored fp16 and upcast on the
    host (rel_err ~5e-4, gate 2e-2). fp16 matmul runs the full-rate
    TensorE path: 8192 tok * 16 blocks * 512 cyc / 2.4 GHz = 54.6us/core;
    HBM traffic ~18.4MB -> ~51us. The kernel sits at the ridge; the
    matmul stream is the critical path, everything else hides under it.
  - Startup: the framework preamble ends ~7us and the first DMA byte
    moves ~8.3us. ALL loads ride the single sync HWDGE ring in exact
    consumption order (w0, c0, bias, c1, ..., w1 after c6, w2 after c8):
    a second active ring makes the SDMA engines round-robin per packet
    and the startup stream crawls, and the scheduler hoists any dep-free
    DMA to its engine's stream head, so ring order is the only reliable
    sequencing. Every x tile is exactly chunk-sized, keeping the SBUF
    destination contiguous (4-8KB descriptors; a strided destination
    shatters into 1KB packets at 1/4 rate). The first two tiles are 256
    tokens so only 768KB gates the first matmul group.
  - HAM warm-up: the PE clock gate sits at 1.2GHz until ~3.4us of
    sustained activity; 22 dummy matmuls on a zeroed scratch tile (N=256,
    tapering to N=128 for a finer drain) bridge PE-busy from the preamble
    end to first-chunk-ready with no idle gap, so the real matmul stream
    runs at 2.4GHz nearly from its first instruction (an idle gap delays
    the un-throttle and the first real tiles then run at 1.2GHz).
  - PSUM->SBUF bias-add alternates DVE / ACT so copies never pace the
    matmuls. Mid-run stores ride the gpsimd SWDGE ring. The last tile
    stores per output block across the rings, and its final block's
    compute+copy is split into two 256-token halves on separate PSUM
    banks (the store stays one 128KB DMA) so only a 256-token copy
    trails the very last matmul.
"""

import sys

sys.path.insert(0, "/opt/trn_rl_repo")

import numpy as np
import concourse.bacc as bacc
import concourse.bass as bass
import concourse.mybir as mybir
import concourse.tile as tile
from concourse.bass_utils import run_bass_kernel_spmd

# run_bass_kernel_spmd imports antenv.axon_hooks when BASS_TRACE is set; some
# images lack that module. Register a no-op fallback so a stray BASS_TRACE
# can only skip profiling, never crash the run.
try:
    import antenv.axon_hooks  # noqa: F401
except ImportError:
    import types

    _hooks = types.ModuleType("antenv.axon_hooks")
    _hooks._hook = None
    _hooks.set_axon_ntff_profile_hook = lambda h: setattr(_hooks, "_hook", h)
    _hooks.get_axon_ntff_profile_hook = lambda: _hooks._hook
    try:
        import antenv

        antenv.axon_hooks = _hooks
        sys.modules["antenv.axon_hooks"] = _hooks
    except ImportError:
        pass

F32 = mybir.dt.float32
F16 = mybir.dt.float16
IDENT = mybir.ActivationFunctionType.Identity

LEN_GROUPS = (256, 512, 768, 384, 640, 512, 576, 448)
NUM_GROUPS, D_IN, D_OUT = 8, 512, 512
BATCH, T = 16, 4096
N_CORES = 8
T_CORE = 8192  # tokens per core (16*4096/8)
KB = D_IN // 128   # 4 contraction blocks
OB = D_OUT // 128  # 4 output blocks
NT = 512           # max moving-dim tile (tokens per matmul)

# Weight slots per core: slot s covers SLOT_SIZES[s] tokens, all of one group.
SLOT_SIZES = (4096, 2560, 1536)
N_SLOTS = 3
# (slot, core) -> group. Tiles the 16*L_g tokens of every group exactly.
SLOT_GROUPS = (
    (0, 1, 1, 2, 2, 2, 6, 7),  # 4096-token slots
    (4, 4, 4, 4, 5, 5, 6, 6),  # 2560-token slots
    (3, 3, 3, 3, 5, 5, 7, 7),  # 1536-token slots
)

# Tiles: small first tiles start compute as early as possible.
TILE_SIZES = [256, 256] + [512] * 15  # 17 tiles, sums to 8192
TILE_STARTS = np.concatenate([[0], np.cumsum(TILE_SIZES)]).tolist()
N_TILES = len(TILE_SIZES)
SLOT_STARTS = np.concatenate([[0], np.cumsum(SLOT_SIZES)]).tolist()

# x staged in chunks; chunk boundaries align with tile boundaries.
CHUNK_SIZES = [256, 256, 512] + [1024] * 6 + [512, 512]
CHUNK_STARTS = np.concatenate([[0], np.cumsum(CHUNK_SIZES)]).tolist()

N_WARM = 22  # dummy warm-up matmuls (N=256 each): bridge PE-busy from the
             # preamble end (~7.6us) to first-chunk-ready (~12.2us) with no
             # idle gap — an idle gap resets the HAM activity window and the
             # first real tiles then run at 1.2GHz

_NC_CACHE = None
_LAST_RESULTS = None  # test harness introspection (exec_time_ns etc.)


def _token_assignment():
    """Per-core global token indices (into x.reshape(-1, 512)), slot-major."""
    starts = np.cumsum((0,) + LEN_GROUPS[:-1])
    pools = []
    for g in range(NUM_GROUPS):
        seg = np.arange(starts[g], starts[g] + LEN_GROUPS[g])
        pools.append(
            (np.arange(BATCH)[:, None] * T + seg[None, :]).reshape(-1)
        )
    used = [0] * NUM_GROUPS
    core_tok = [[] for _ in range(N_CORES)]
    for s in range(N_SLOTS):
        size = SLOT_SIZES[s]
        for c in range(N_CORES):
            g = SLOT_GROUPS[s][c]
            core_tok[c].append(pools[g][used[g]:used[g] + size])
            used[g] += size
    assert all(used[g] == BATCH * LEN_GROUPS[g] for g in range(NUM_GROUPS))
    return [np.concatenate(t) for t in core_tok]


TOKEN_INDEX = _token_assignment()


def _build_nc():
    nc = bacc.Bacc("TRN2", target_bir_lowering=False, debug=False,
                   num_devices=N_CORES)

    # All buffers packed in exact DMA consumption order (sequential HBM).
    xP = nc.dram_tensor("xP", [D_IN * T_CORE], F16, kind="ExternalInput").ap()
    wP = nc.dram_tensor("wP", [N_SLOTS * D_IN * D_OUT], F16,
                        kind="ExternalInput").ap()
    bS = nc.dram_tensor("bS", [128, N_SLOTS * OB], F32,
                        kind="ExternalInput").ap()
    oP = nc.dram_tensor("oP", [D_OUT * T_CORE], F16, kind="ExternalOutput").ap()

    w_len = D_IN * D_OUT

    with tile.TileContext(nc) as tc:
        with (
            tc.tile_pool(name="wpool", bufs=1) as wpool,
            tc.tile_pool(name="bpool", bufs=1) as bpool,
            tc.tile_pool(name="warmp", bufs=1) as warmp,
            tc.tile_pool(name="xpool", bufs=4) as xpool,
            tc.tile_pool(name="opool", bufs=4) as opool,
            tc.tile_pool(name="psum", bufs=8, space=bass.MemorySpace.PSUM) as psum,
        ):
            # Weights resident in SBUF: [p, s, k, o] = W^T[g_s][k*128+p, o]
            w_sb = wpool.tile([128, N_SLOTS, KB, D_OUT], F16)
            b_sb = bpool.tile([128, N_SLOTS * OB], F32)

            # sync ring: w0 then every x chunk, in consumption order. Keep
            # the scalar ring SILENT at startup: a second active ring makes
            # the SDMA engines round-robin per packet and the startup
            # stream crawls (measured 2-5x slowdown on the critical loads).
            nc.sync.dma_start(
                w_sb[:, 0, :, :],
                wP[0:w_len].rearrange("(p k o) -> p k o", p=128, k=KB),
            )

            # HAM warm-up: keep the PE busy while the first loads stream
            # in so the clock gate lifts to 2.4GHz as early as possible.
            # The memset rides gpsimd (the earliest-ready engine) so the
            # dummies start the moment the Tensor engine comes up; the
            # last few dummies are N=128 so the queue drains at a finer
            # granularity once the real data lands.
            warm_sb = warmp.tile([128, 256], F16)
            nc.gpsimd.memset(warm_sb[:], 0.0)
            warm_acc = psum.tile([128, NT], F32, tag="acc")
            for j in range(N_WARM):
                wn = 256 if j < N_WARM - 8 else 128
                nc.tensor.matmul(
                    warm_acc[:, 0:wn], warm_sb[:, 0:128], warm_sb[:, 0:wn],
                    start=True, stop=True,
                )

            x_chunks = [None] * len(CHUNK_SIZES)
            o_off = 0
            for ti in range(N_TILES):
                t0, ntok = TILE_STARTS[ti], TILE_SIZES[ti]
                s = next(
                    j for j in range(N_SLOTS)
                    if SLOT_STARTS[j] <= t0 < SLOT_STARTS[j + 1]
                )
                ci = next(
                    j for j in range(len(CHUNK_SIZES))
                    if CHUNK_STARTS[j] <= t0 < CHUNK_STARTS[j + 1]
                )
                co = t0 - CHUNK_STARTS[ci]  # offset within chunk
                if x_chunks[ci] is None:
                    csz = CHUNK_SIZES[ci]
                    # exact-size tile: contiguous SBUF dest -> 4-8KB DMA
                    # descriptors (a strided dest shatters into 1KB packets)
                    x_sb = xpool.tile(
                        [128, KB, csz], F16,
                        tag=f"x{csz}",
                        bufs=2 if csz < 1024 else 4,
                    )
                    nc.sync.dma_start(
                        x_sb[:],
                        xP[CHUNK_STARTS[ci] * D_IN:CHUNK_STARTS[ci + 1] * D_IN]
                        .rearrange("(p k t) -> p k t", p=128, k=KB),
                    )
                    x_chunks[ci] = x_sb
                    if ci == 0:
                        # bias (128 tiny descriptors) queues behind c0 on
                        # the same ring; it is only needed by the first copy
                        nc.sync.dma_start(b_sb[:], bS)
                    elif ci == 6:
                        # w1 queues once the x stream has a multi-chunk
                        # lead; first needed at tile 9 (~15us later)
                        nc.sync.dma_start(
                            w_sb[:, 1, :, :],
                            wP[w_len:2 * w_len]
                            .rearrange("(p k o) -> p k o", p=128, k=KB),
                        )
                    elif ci == 8:
                        nc.sync.dma_start(
                            w_sb[:, 2, :, :],
                            wP[2 * w_len:3 * w_len]
                            .rearrange("(p k o) -> p k o", p=128, k=KB),
                        )
                x_sb = x_chunks[ci]

                last = ti == N_TILES - 1
                o_sb = opool.tile(
                    [128, OB, ntok], F16, tag=f"o{ntok}",
                    bufs=2 if ntok == 256 else 4,
                )
                o_len = 128 * OB * ntok
                o_dram = oP[o_off:o_off + o_len].rearrange(
                    "(p ob t) -> p ob t", p=128, ob=OB
                )
                o_off += o_len

                # copy engines alternate DVE ("v") / ACT ("s"); on the last
                # tile the final block rides DVE and stores fan per-block
                copy_eng = ("v", "s", "s", "v") if last else ("v", "s", "v", "s")
                store_eng = (nc.gpsimd, nc.gpsimd, nc.sync, nc.scalar)

                for ob in range(OB):
                    bias_ap = b_sb[:, s * OB + ob:s * OB + ob + 1]
                    w_ap = w_sb[:, s, :, ob * 128:(ob + 1) * 128]
                    if last and ob == OB - 1:
                        # split the final block's compute+copy into two
                        # 256-token halves on separate PSUM banks (the first
                        # half's copy hides under the second half's matmuls,
                        # so only a 256-token copy trails the last matmul),
                        # but store the block as one 128KB DMA
                        for h in range(2):
                            ho = h * 256
                            acc = psum.tile([128, NT], F32, tag="acc")
                            for k in range(KB):
                                nc.tensor.matmul(
                                    acc[:, 0:256],
                                    w_ap[:, k, :],
                                    x_sb[:, k, co + ho:co + ho + 256],
                                    start=(k == 0),
                                    stop=(k == KB - 1),
                                )
                            nc.vector.tensor_scalar_add(
                                o_sb[:, ob, ho:ho + 256], acc[:, 0:256],
                                bias_ap,
                            )
                        nc.scalar.dma_start(
                            o_dram[:, ob:ob + 1, :], o_sb[:, ob:ob + 1, :]
                        )
                        continue
                    acc = psum.tile([128, NT], F32, tag="acc")
                    for k in range(KB):
                        nc.tensor.matmul(
                            acc[:, 0:ntok],
                            w_ap[:, k, :],
                            x_sb[:, k, co:co + ntok],
                            start=(k == 0),
                            stop=(k == KB - 1),
                        )
                    if copy_eng[ob] == "v":
                        nc.vector.tensor_scalar_add(
                            o_sb[:, ob, :], acc[:, 0:ntok], bias_ap
                        )
                    else:
                        nc.scalar.activation(
                            o_sb[:, ob, :], acc[:, 0:ntok], IDENT, bias=bias_ap
                        )
                    if last:
                        store_eng[ob].dma_start(
                            o_dram[:, ob:ob + 1, :], o_sb[:, ob:ob + 1, :]
                        )

                if ti == N_TILES - 2:
                    # second-to-last tile: split across gpsimd + sync (the
                    # sync ring is done loading by now; scalar stays free
                    # for the final tile's ACT copies)
                    nc.gpsimd.dma_start(o_dram[:, 0:2, :], o_sb[:, 0:2, :])
                    nc.sync.dma_start(o_dram[:, 2:4, :], o_sb[:, 2:4, :])
                elif not last:
                    # mid-run stores ride the gpsimd SWDGE ring, keeping
                    # compute-dependent instructions off the x ring
                    nc.gpsimd.dma_start(o_dram, o_sb[:])

    nc.compile()
    return nc


def kernel(x: np.ndarray, W: np.ndarray, b: np.ndarray) -> np.ndarray:
    global _NC_CACHE, _LAST_RESULTS
    x = np.asarray(x, dtype=np.float32)
    W = np.asarray(W, dtype=np.float32)
    b = np.asarray(b, dtype=np.float32)

    if _NC_CACHE is None:
        _NC_CACHE = _build_nc()
    nc = _NC_CACHE

    wT = np.ascontiguousarray(W.transpose(0, 2, 1)).astype(np.float16)  # [g,d,o]
    x_flat = x.reshape(BATCH * T, D_IN)

    in_maps = []
    for c in range(N_CORES):
        groups = [SLOT_GROUPS[s][c] for s in range(N_SLOTS)]
        # wP packed [s][p][k][o] = wT[g_s][k*128+p, o]
        wsel = wT[groups]  # [3, 512, 512] = [s, (k p), o]
        wP = np.ascontiguousarray(
            wsel.reshape(N_SLOTS, KB, 128, D_OUT).transpose(0, 2, 1, 3)
        ).reshape(-1)
        # bias laid out [p, s*4 + ob] = b[g_s, ob*128 + p]
        bS = np.ascontiguousarray(
            b[groups].reshape(N_SLOTS, OB, 128).transpose(2, 0, 1)
            .reshape(128, N_SLOTS * OB)
        )
        # xP packed per chunk as [p][k][t]: (p,k,t) = x^T[k*128+p, chunk+t]
        xc = x_flat[TOKEN_INDEX[c]].astype(np.float16)  # [8192, 512]
        parts = []
        for j, csz in enumerate(CHUNK_SIZES):
            t0, t1 = CHUNK_STARTS[j], CHUNK_STARTS[j + 1]
            blk = xc[t0:t1].T  # [512 d, csz]
            parts.append(
                np.ascontiguousarray(
                    blk.reshape(KB, 128, csz).transpose(1, 0, 2)
                ).reshape(-1)
            )
        xP = np.concatenate(parts)
        in_maps.append({"xP": xP, "wP": wP, "bS": bS})

    res = run_bass_kernel_spmd(nc, in_maps, list(range(N_CORES)))
    _LAST_RESULTS = res

    out = np.empty((BATCH * T, D_OUT), dtype=np.float32)
    for c in range(N_CORES):
        oc_flat = res.results[c]["oP"]
        rows = []
        off = 0
        for ntok in TILE_SIZES:
            seg = oc_flat[off:off + 128 * OB * ntok].reshape(128, OB, ntok)
            # [p, ob, t] -> [t, (ob p) = o]
            rows.append(seg.transpose(2, 1, 0).reshape(ntok, D_OUT))
            off += 128 * OB * ntok
        oc = np.concatenate(rows).astype(np.float32)  # [8192, 512]
        out[TOKEN_INDEX[c]] = oc
    return out.reshape(BATCH, T, D_OUT)
